# revision 13
# baseline (speedup 1.0000x reference)
# Trainium2 Bass kernel for nn_Action2 — row-space redesign.
# Data-parallel across 8 NeuronCores: batch 8192 -> 1024 per core, split into
# TWO 512-wide streams in anti-phase.  x is kept in ROW space as feature row 0
# of the staging tile, so the per-step chain is
#   mm1(K=15) -> relu1 -> mm2 -> relu2 -> mm3 -> exp(row) -> g(row) -> xmul(row)
# with NO DVE transposes and no [32,16] update domain.  Engine APs need
# quadrant-aligned partition bases, so pi lands on PSUM row 0 and lc on PSUM
# row 32 (w3 spread over cols 0/32); per-step drift rows and the x history
# travel via DMA (which has no partition-alignment limits).
import os
import sys

import numpy as np

for _p in ("/opt/trn_rl_repo",):
    if _p not in sys.path:
        sys.path.insert(0, _p)

import concourse.bacc as bacc  # noqa: E402
import concourse.mybir as mybir  # noqa: E402
import concourse.tile as tile  # noqa: E402
from concourse.bass_utils import run_bass_kernel_spmd  # noqa: E402
from concourse.tile_rust import add_dep_helper  # noqa: E402

F32 = mybir.dt.float32
F16 = mybir.dt.float16
ALU = mybir.AluOpType
ACTF = mybir.ActivationFunctionType

B_GLOBAL = 8192
N_CORES = 8
B = B_GLOBAL // N_CORES   # 1024 per core
SB = B // 2               # 512 per stream
N_STEPS = 100
IN_DIM = 5
T_HORIZON = 1.0
MU, NU, SIGMA = 0.1, 0.2, 0.3
BN_EPS = 1e-5
DT = T_HORIZON / N_STEPS

# stg rows: 0 x_i, 1-5 bn, 6-10 bnc, 11 ones, 12 mx_i, 13 mc_i, 14 t_i.


def build(n_steps=N_STEPS):
    nc = bacc.Bacc("TRN2", target_bir_lowering=False, debug=False)

    def din(name, shape, dtype):
        return nc.dram_tensor(name, list(shape), dtype, kind="ExternalInput").ap()

    STATIC11 = din("static11", (11, B), F16)      # bn/bnc/ones rows
    MXMCT = din("mxmct", (n_steps, 3, B), F16)    # mx, mc, t rows
    DROW = din("drow", (n_steps, B), F32)         # per-step drift rows
    B2EFF = din("b2eff", (128, 1), F32)
    W1R_D = din("w1r", (15, 128), F16)
    W2S_D = din("w2s", (128, 128), F16)
    W3S_D = din("w3s", (128, 33), F16)            # col0=w3 (pi), col32=wc3 (lc)
    SCAL = din("scal", (1, 2), F32)               # [b3, bc3+log(dt)]
    X0 = din("x0", (1, B), F16)

    OUT = nc.dram_tensor("out", [n_steps + 1, B], F16,
                         kind="ExternalOutput").ap()

    with tile.TileContext(nc) as tc:
        import contextlib

        with contextlib.ExitStack() as ctx:
            const = ctx.enter_context(tc.tile_pool(name="const", bufs=1))
            stgp = ctx.enter_context(tc.tile_pool(name="stg", bufs=1))
            h1p = ctx.enter_context(tc.tile_pool(name="h1", bufs=2))
            h2p = ctx.enter_context(tc.tile_pool(name="h2", bufs=2))
            updp = ctx.enter_context(tc.tile_pool(name="upd", bufs=2))
            ps1 = ctx.enter_context(tc.tile_pool(name="ps1", bufs=1, space="PSUM"))
            ps2 = ctx.enter_context(tc.tile_pool(name="ps2", bufs=1, space="PSUM"))
            ps3 = ctx.enter_context(tc.tile_pool(name="ps3", bufs=1, space="PSUM"))

            w1r = const.tile([15, 128], F16)
            nc.sync.dma_start(w1r[:], W1R_D)
            w2r = const.tile([128, 128], F16)
            nc.sync.dma_start(w2r[:], W2S_D)
            w3b = const.tile([128, 33], F16)
            nc.sync.dma_start(w3b[:], W3S_D)
            b2eff = const.tile([128, 1], F32)
            nc.sync.dma_start(b2eff[:], B2EFF)
            scal = const.tile([1, 2], F32)
            nc.sync.dma_start(scal[:], SCAL)
            b3s = scal[0:1, 0:1]
            bc3s = scal[0:1, 1:2]

            # rotating per-step drift rows: slot i%3, stream s at cols 512*s
            dtile = const.tile([1, 3 * B], F32)
            nc.sync.dma_start(dtile[0:1, 0:B], DROW[0:1].rearrange("a b -> (a b)"))
            if n_steps > 1:
                nc.sync.dma_start(dtile[0:1, B:2 * B],
                                  DROW[1:2].rearrange("a b -> (a b)"))

            stg = [stgp.tile([15, B], F16, tag=f"stgt{k}", name=f"stgt{k}")
                   for k in range(3)]
            for k in range(3):
                nc.sync.dma_start(stg[k][1:12, :], STATIC11)
            nc.sync.dma_start(stg[0][12:15, :], MXMCT[0])
            if n_steps > 1:
                nc.sync.dma_start(stg[1][12:15, :], MXMCT[1])
            nc.sync.dma_start(stg[0][0:1, :], X0)

            h1_live = {}
            h2_live = {}
            skew_anchor = [None]

            def emit_head1(s, i):
                # L1 -> relu1 for stream s, step i
                st = stg[i % 3]
                p1 = ps1.tile([128, SB], F32, tag=f"p1{s}", name=f"p1_{s}_{i}")
                mm1 = nc.tensor.matmul(p1[:], w1r[:],
                                       st[:, SB * s:SB * (s + 1)],
                                       start=True, stop=True)
                if i == 0 and s == 1 and skew_anchor[0] is not None:
                    add_dep_helper(mm1.ins, skew_anchor[0], sync=True,
                                   reason="stream anti-phase skew")
                h1 = h1p.tile([128, SB], F16, tag=f"h1{s}", name=f"h1_{s}_{i}")
                nc.scalar.activation(h1[:], p1[:], ACTF.Relu)
                h1_live[s] = h1

            def emit_head2(s, i):
                # L2 -> relu2 for stream s, step i
                p2 = ps2.tile([128, SB], F32, tag=f"p2{s}", name=f"p2_{s}_{i}")
                nc.tensor.matmul(p2[:], w2r[:], h1_live[s][:],
                                 start=True, stop=True)
                h2 = h2p.tile([128, SB], F16, tag=f"h2{s}", name=f"h2_{s}_{i}")
                r2 = nc.scalar.activation(h2[:], p2[:], ACTF.Relu,
                                          bias=b2eff[:])
                if i == 0 and s == 0:
                    skew_anchor[0] = r2.ins
                h2_live[s] = h2

            def emit_tail(s, i):
                # L3 -> exp/a/g/xmul in row space for stream s, step i
                p3 = ps3.tile([33, SB], F32, tag=f"p3{s}", name=f"p3_{s}_{i}")
                nc.tensor.matmul(p3[:], w3b[:], h2_live[s][:],
                                 start=True, stop=True)
                cdt = updp.tile([1, SB], F16, tag=f"cdt{s}", name=f"cdt_{s}_{i}")
                nc.scalar.activation(cdt[:], p3[32:33, :], ACTF.Exp, bias=bc3s)
                a_t = updp.tile([1, SB], F16, tag=f"a{s}", name=f"a_{s}_{i}")
                u_t = updp.tile([1, SB], F16, tag=f"u{s}", name=f"u_{s}_{i}")
                dsl = dtile[0:1, B * (i % 3) + SB * s:B * (i % 3) + SB * (s + 1)]
                nc.vector.scalar_tensor_tensor(
                    a_t[:], p3[0:1, :], b3s, dsl, ALU.add, ALU.mult)
                # u = a - c*dt (all fp16 SBUF, DVE fast mode eligible);
                # xnext = (u + 1) * x
                nc.vector.tensor_sub(u_t[:], a_t[:], cdt[:])
                xcur = stg[i % 3][0:1, SB * s:SB * (s + 1)]
                xnext = stg[(i + 1) % 3][0:1, SB * s:SB * (s + 1)]
                nc.vector.scalar_tensor_tensor(
                    xnext, u_t[:], 1.0, xcur, ALU.add, ALU.mult)

            # Software-pipelined anti-phase emission: per-engine program order
            # matches op readiness under a half-period stream skew.
            for i in range(n_steps):
                emit_head1(0, i)
                if i > 0:
                    emit_tail(1, i - 1)
                    # history: x_i (both streams) -> OUT row i; depends on
                    # xmul(0, i-1) [prev iter] and xmul(1, i-1) [just above]
                    nc.sync.dma_start(OUT[i:i + 1, :], stg[i % 3][0:1, :])
                if i + 2 < n_steps:
                    nc.sync.dma_start(stg[(i + 2) % 3][12:15, :], MXMCT[i + 2])
                    nc.sync.dma_start(
                        dtile[0:1, B * ((i + 2) % 3):B * ((i + 2) % 3) + B],
                        DROW[i + 2:i + 3].rearrange("a b -> (a b)"))
                emit_head2(0, i)
                emit_head1(1, i)
                emit_tail(0, i)
                emit_head2(1, i)
            emit_tail(1, n_steps - 1)
            nc.sync.dma_start(OUT[n_steps:n_steps + 1, :],
                              stg[n_steps % 3][0:1, :])

    nc.compile()
    return nc


def host_prep(inputs, n_steps=N_STEPS):
    F16_NP = np.float16
    bm = np.asarray(inputs["bm"], np.float32)
    cn = np.asarray(inputs["cn"], np.float32)
    typeVec = np.asarray(inputs["typeVec"], np.float32)
    mx = np.asarray(inputs["mx"], np.float32)
    mc = np.asarray(inputs["mc"], np.float32)
    initial = float(np.asarray(inputs["initial"]).reshape(-1)[0])
    bn_gamma = np.asarray(inputs["bn_gamma"], np.float32)
    bn_beta = np.asarray(inputs["bn_beta"], np.float32)
    bnc_gamma = np.asarray(inputs["bnc_gamma"], np.float32)
    bnc_beta = np.asarray(inputs["bnc_beta"], np.float32)
    w1 = np.asarray(inputs["w1"], np.float32)
    b1 = np.asarray(inputs["b1"], np.float32)
    w2 = np.asarray(inputs["w2"], np.float32)
    b2 = np.asarray(inputs["b2"], np.float32)
    w3 = np.asarray(inputs["w3"], np.float32)
    b3 = np.asarray(inputs["b3"], np.float32)
    wc1 = np.asarray(inputs["wc1"], np.float32)
    bc1 = np.asarray(inputs["bc1"], np.float32)
    wc2 = np.asarray(inputs["wc2"], np.float32)
    bc2 = np.asarray(inputs["bc2"], np.float32)
    wc3 = np.asarray(inputs["wc3"], np.float32)
    bc3 = np.asarray(inputs["bc3"], np.float32)

    Bg, N, _ = bm.shape
    assert Bg == B_GLOBAL and N >= n_steps

    m = typeVec.mean(axis=0, dtype=np.float64)
    v = ((typeVec.astype(np.float64) - m) ** 2).mean(axis=0)
    inv = 1.0 / np.sqrt(v + BN_EPS)
    bn = ((typeVec - m) * inv * bn_gamma + bn_beta).astype(np.float32)
    bnc = ((typeVec - m) * inv * bnc_gamma + bnc_beta).astype(np.float32)

    dcn = cn[:, 1:n_steps + 1, 0] - cn[:, :n_steps, 0]
    drift = (np.float32(MU * DT) + np.float32(NU) * bm[:, :n_steps, 0]
             + np.float32(SIGMA) * dcn).astype(np.float32)
    mxs = mx[:, :n_steps, 0]
    mcs = mc[:, :n_steps, 0]
    ts = (np.arange(n_steps, dtype=np.float32) * np.float32(DT))

    def stack_row(a, b_):
        return np.concatenate([a, b_]).astype(np.float32)

    w1r = np.zeros((15, 128), np.float32)
    w1r[0] = stack_row(w1[6], wc1[6])     # x
    for k in range(5):
        w1r[1 + k, 0:64] = w1[k]
        w1r[6 + k, 64:128] = wc1[k]
    w1r[11] = stack_row(b1, bc1)          # ones row -> layer-1 bias
    w1r[12] = stack_row(w1[7], wc1[7])    # mx
    w1r[13] = stack_row(w1[8], wc1[8])    # mc
    w1r[14] = stack_row(w1[5], wc1[5])    # t

    b2eff = np.concatenate([b2, bc2]).astype(np.float32).reshape(128, 1)
    w2s = np.zeros((128, 128), np.float32)
    w2s[0:64, 0:64] = w2
    w2s[64:128, 64:128] = wc2
    w3s = np.zeros((128, 33), np.float32)
    w3s[0:64, 0] = w3[:, 0]
    w3s[64:128, 32] = wc3[:, 0]
    scal = np.zeros((1, 2), np.float32)
    scal[0, 0] = b3[0]
    scal[0, 1] = np.float32(bc3[0] + np.log(DT))

    in_maps = []
    for core in range(N_CORES):
        sl = slice(core * B, (core + 1) * B)
        bn_c, bnc_c = bn[sl], bnc[sl]
        static11 = np.empty((11, B), np.float32)
        static11[0:5] = bn_c.T
        static11[5:10] = bnc_c.T
        static11[10] = 1.0
        mxmct = np.empty((n_steps, 3, B), np.float32)
        mxmct[:, 0, :] = mxs[sl].T
        mxmct[:, 1, :] = mcs[sl].T
        mxmct[:, 2, :] = ts[:, None]
        d_np = np.ascontiguousarray(drift[sl].T)   # (n_steps, 1024)
        x0 = np.full((1, B), initial, np.float32)
        in_maps.append({
            "static11": static11.astype(F16_NP),
            "mxmct": mxmct.astype(F16_NP),
            "drow": d_np,
            "b2eff": b2eff.copy(),
            "w1r": w1r.astype(F16_NP),
            "w2s": w2s.astype(F16_NP),
            "w3s": w3s.astype(F16_NP),
            "scal": scal.copy(),
            "x0": x0.astype(F16_NP),
        })
    return in_maps, initial


def assemble_output(results, initial, n_steps=N_STEPS):
    states = np.empty((B_GLOBAL, n_steps + 1), np.float32)
    for core in range(N_CORES):
        out = results[core]["out"]              # (n_steps+1, 1024) fp16
        states[core * B:(core + 1) * B, 1:] = out[1:n_steps + 1].astype(np.float32).T
    states[:, 0] = initial
    times = (np.arange(n_steps + 1, dtype=np.float32) * np.float32(DT))
    full = np.empty((B_GLOBAL, n_steps + 1, 2), np.float32)
    full[:, :, 0] = times[None, :]
    full[:, :, 1] = states
    return full


_BUILT = {}


def _get_built(n_steps=N_STEPS):
    if n_steps not in _BUILT:
        _BUILT[n_steps] = build(n_steps)
    return _BUILT[n_steps]


def kernel(**inputs):
    nc = _get_built()
    in_maps, initial = host_prep(inputs)
    res = run_bass_kernel_spmd(nc, in_maps, core_ids=list(range(N_CORES)))
    return assemble_output(res.results, initial)


if __name__ == "__main__":
    sys.path.insert(0, os.path.dirname(os.path.abspath(__file__)))
    import reference

    inputs = reference.setup_inputs()
    inputs = {k: np.asarray(v) for k, v in inputs.items()}
    expected = np.asarray(reference.reference(**inputs))
    actual = kernel(**inputs)
    err = np.abs(actual - expected)
    print("max abs err:", err.max())
    print("rel err (scale):", err.max() / np.abs(expected).max())
